# revision 19
# baseline (speedup 1.0000x reference)
"""Trainium2 Bass kernel for leave-one-out Nadaraya-Watson regression
(nn_Net_72877005078649) — fast-Gauss-transform formulation, v3.

Per output channel o this is 1D Gaussian kernel regression; the kernel
factorizes through a G=12 grid (a = b = h/sqrt(2), trapezoid aliasing
~1e-4):  K_h(x,z) ~= kappa * sum_g exp(-(c_g-x)^2/h^2) exp(-(z-c_g)^2/h^2)

v3 design notes (vs v2 baseline at ~41.5us):
 - per-core input roll: core c's train data is rotated so its own 512
   queries are train chunk 0 — the query-side Eq is just cols 0:512 of
   pair-0's train-side exp table ET0. The whole separate query
   projection chain (W1/relu/W2/sub/sq/exp on [*,512]) is gone.
 - inputs ship packed on 128 partitions, pair-contiguous: tX_q
   [128,512] bf16 holds two 512-col train chunks stacked on partition
   halves (W1T duplicated on rows 0:64 and 64:128 of wpack so both
   halves matmul with base-partition-aligned lhsT). Few large
   contiguous DMAs across 4 engine queues instead of many tiny-packet
   strided ones.
 - finalize: fin_j [128q,20] = matmul(lhsT=ET0[:, j*128:+128] bf16,
   rhs=AA bf16) gives num|den directly in query-partition layout — no
   identity transposes, no PSUM->SBUF copy, no make_identity.
 - elementwise rebalance per pair across ACT/DVE/GpSimd; ACT exp keeps
   den via accum_out, num via STT (DVE on odd / GpSimd on even pairs).
 - diagonal: train_X == x by construction, K_ii == 1 exactly:
   out = (num - Y_d)/(den - 1).

Sharding: queries split across 8 cores (512/core); train replicated.
"""

import numpy as np

N = 4096
D = 64
HID = 128
O = 10
NCORES = 8
BQ = N // NCORES
G = 12
GRID_LO = -6.5
GRID_HI = 6.5
NPAIR = 4               # train chunk pairs, 1024 cols each

_cache = {}


def _host_consts(h: float):
    c = np.linspace(GRID_LO, GRID_HI, G).astype(np.float32)
    delta = float(c[1] - c[0])
    kappa = 2.0 * delta / (np.sqrt(2.0 * np.pi) * h)
    # cbase[128, 22] = cq[128,1] | kmask2[128,20] | -cq[128,1]
    # ET is computed via Derivative_Erf = (2/sqrt(pi))*exp(-u^2); both the
    # query and train factors carry 2/sqrt(pi), so fold (pi/4) into kappa.
    kap = kappa * np.pi / 4.0
    cbase = np.zeros((128, 22), np.float32)
    for p in range(128):
        cbase[p, 0] = c[min(p // O, G - 1)]
        cbase[p, 21] = -c[min(p // O, G - 1)] / h   # DErf bias: -c_g/h
    for p in range(G * O):
        cbase[p, 1 + p % O] = kap            # num mask
        cbase[p, 11 + p % O] = kap           # den mask
    return cbase


def _host_yext(Y, c):
    # yext[p, j*20+e] = Y[c*512 + j*128 + p, e] for e<10, 1.0 for e>=10;
    # lets the finalize subtract num-Y and den-1 in a single tensor_tensor.
    yext = np.ones((128, 4, 2 * O), np.float32)
    yext[:, :, 0:O] = Y[c * BQ:(c + 1) * BQ].reshape(4, 128, O).transpose(1, 0, 2)
    return yext.reshape(128, 4 * 2 * O)


def _build(h: float):
    import concourse.bass as bass
    import concourse.bacc as bacc
    import concourse.tile as tile
    from concourse import mybir

    f32 = mybir.dt.float32
    bf16 = mybir.dt.bfloat16
    AF = mybir.ActivationFunctionType
    ALU = mybir.AluOpType

    inv_h = 1.0 / h

    nc = bacc.Bacc("TRN2", target_bir_lowering=False, debug=False, num_devices=1)
    tXd = [nc.dram_tensor(f"tX{q}", [128, 512], bf16, kind="ExternalInput").ap()
           for q in range(NPAIR)]
    Yrd = [nc.dram_tensor(f"Yr{q}", [128, 1024], bf16, kind="ExternalInput").ap()
           for q in range(NPAIR)]
    wpd = nc.dram_tensor("wpack", [128, 256], bf16, kind="ExternalInput").ap()
    cpd = nc.dram_tensor("cpack", [128, 102], f32, kind="ExternalInput").ap()
    out = nc.dram_tensor("out", [BQ, O], f32, kind="ExternalOutput").ap()

    with tile.TileContext(nc) as tc:
        with (
            tc.tile_pool(name="S", bufs=1) as S,
            tc.tile_pool(name="W", bufs=2) as W,
            tc.tile_pool(name="PS", bufs=1, space="PSUM") as PS,
        ):
            # ---- ACT warmup (loads the multi-func table once) ----
            warm = S.tile([1, 16], f32)
            nc.vector.memset(warm, 0.0)
            nc.scalar.activation(out=warm, in_=warm, func=AF.Derivative_Erf)

            # ---- input DMAs: spread across engine queues, big+contiguous
            wp = S.tile([128, 256], bf16)
            cp = S.tile([128, 102], f32)
            tX = [S.tile([128, 512], bf16, name=f"tX{q}") for q in range(NPAIR)]
            Yr = [S.tile([128, 1024], bf16, name=f"Yr{q}") for q in range(NPAIR)]
            # within-queue FIFO = strict priority; keep the big Yr tables
            # behind the critical weights + train chunks
            nc.sync.dma_start(out=tX[0], in_=tXd[0])
            nc.scalar.dma_start(out=wp, in_=wpd)
            nc.sync.dma_start(out=tX[1], in_=tXd[1])
            nc.scalar.dma_start(out=cp, in_=cpd)
            nc.sync.dma_start(out=tX[2], in_=tXd[2])
            nc.sync.dma_start(out=tX[3], in_=tXd[3])
            nc.gpsimd.dma_start(out=Yr[2], in_=Yrd[2])
            nc.gpsimd.dma_start(out=Yr[3], in_=Yrd[3])
            nc.sync.dma_start(out=Yr[0], in_=Yrd[0])
            nc.scalar.dma_start(out=Yr[1], in_=Yrd[1])

            w1a = wp[0:64, 0:128]
            w1b = wp[64:128, 0:128]
            w2r = wp[:, 128:256]
            kmask2 = cp[:, 1:21]
            ncq = cp[:, 21:22]
            yext = cp[:, 22:102]

            nparts = S.tile([128, NPAIR + 1], f32)
            dparts = S.tile([128, NPAIR + 1], f32)
            parts = S.tile([128, 2], f32)
            AA = S.tile([128, 2 * O], bf16)
            ET0 = S.tile([128, 1024], bf16)
            nall = S.tile([128, 4 * 2 * O], f32)
            rsb = S.tile([128, 4 * O], f32)
            osb = S.tile([128, 4 * O], f32)

            # ---- 4 train pairs, software-pipelined ----
            hps_t = []

            def w1_pair(q):
                hps = PS.tile([128, 1024], f32, tag="hps", bufs=2,
                              name=f"hps{q}")
                hps_t.append(hps)
                nc.tensor.matmul(hps[:, 0:512], lhsT=w1a, rhs=tX[q][0:64, :],
                                 start=True, stop=True)
                nc.tensor.matmul(hps[:, 512:1024], lhsT=w1b,
                                 rhs=tX[q][64:128, :],
                                 start=True, stop=True)

            w1_pair(0)
            for q in range(NPAIR):
                last = q == NPAIR - 1
                h1 = W.tile([128, 1024], bf16, tag="h1", bufs=3)
                if last:
                    # split halves: shorter serial drain into the finalize
                    nc.vector.tensor_scalar(out=h1[:, 0:512],
                                            in0=hps_t[q][:, 0:512],
                                            scalar1=0.0, scalar2=None,
                                            op0=ALU.max)
                    nc.vector.tensor_scalar(out=h1[:, 512:1024],
                                            in0=hps_t[q][:, 512:1024],
                                            scalar1=0.0, scalar2=None,
                                            op0=ALU.max)
                elif q == 1:
                    # one relu on ACT to balance engine load
                    nc.scalar.activation(out=h1, in_=hps_t[q], func=AF.Relu)
                else:
                    nc.vector.tensor_scalar(out=h1, in0=hps_t[q], scalar1=0.0,
                                            scalar2=None, op0=ALU.max)
                if q + 1 < NPAIR:
                    w1_pair(q + 1)
                xr = PS.tile([128, 1024], f32, tag="xr", bufs=2, name=f"xr{q}")
                nc.tensor.matmul(xr[:, 0:512], lhsT=w2r, rhs=h1[:, 0:512],
                                 start=True, stop=True)
                nc.tensor.matmul(xr[:, 512:1024], lhsT=w2r, rhs=h1[:, 512:1024],
                                 start=True, stop=True)
                # fused Gaussian: DErf(xr/h - c_g/h) = 2/sqrt(pi) exp(-s(xr-c)^2)
                ET = ET0 if q == 0 else W.tile([128, 1024], bf16, tag="ET",
                                               bufs=3)
                scr = W.tile([128, 1024], bf16, tag="scr", bufs=3)
                if last:
                    for hh in range(2):
                        sl = slice(hh * 512, (hh + 1) * 512)
                        nc.scalar.activation(out=ET[:, sl], in_=xr[:, sl],
                                             func=AF.Derivative_Erf,
                                             bias=ncq, scale=inv_h,
                                             accum_out=dparts[:, q + hh:q + hh + 1])
                        nc.vector.scalar_tensor_tensor(
                            out=scr[:, sl], in0=ET[:, sl], scalar=1.0,
                            in1=Yr[q][:, sl],
                            op0=ALU.bypass, op1=ALU.mult,
                            accum_out=nparts[:, q + hh:q + hh + 1])
                else:
                    nc.scalar.activation(out=ET, in_=xr,
                                         func=AF.Derivative_Erf,
                                         bias=ncq, scale=inv_h,
                                         accum_out=dparts[:, q:q + 1])
                    nc.vector.scalar_tensor_tensor(
                        out=scr, in0=ET, scalar=1.0, in1=Yr[q],
                        op0=ALU.bypass, op1=ALU.mult,
                        accum_out=nparts[:, q:q + 1])

            # ---- tables -> AA (bf16 for the bf16 fin matmuls) ----
            nc.vector.tensor_reduce(out=parts[:, 0:1], in_=nparts,
                                    axis=mybir.AxisListType.X, op=ALU.add)
            nc.vector.tensor_reduce(out=parts[:, 1:2], in_=dparts,
                                    axis=mybir.AxisListType.X, op=ALU.add)
            PP = parts.ap[0][0]
            parts_b = bass.AP(tensor=parts.tensor, offset=parts.offset,
                              ap=[[PP, 128], [1, 2], [0, O]])
            nc.vector.tensor_tensor(out=AA.rearrange("p (k e) -> p k e", e=O),
                                    in0=parts_b,
                                    in1=kmask2.rearrange("p (k e) -> p k e", e=O),
                                    op=ALU.mult)

            # ---- query contraction directly into query-partition layout
            fin = PS.tile([128, 4 * 2 * O], f32, tag="xr", bufs=2, name="fin")
            for j in range(4):
                nc.tensor.matmul(fin[:, j * 20:(j + 1) * 20],
                                 lhsT=ET0[:, j * 128:(j + 1) * 128], rhs=AA,
                                 start=True, stop=True)

            # single subtract: num-Y and den-1 at once (yext = Y | ones)
            nc.vector.tensor_tensor(out=nall, in0=fin, in1=yext,
                                    op=ALU.subtract)
            NP = nall.ap[0][0]
            numv = bass.AP(tensor=nall.tensor, offset=nall.offset,
                           ap=[[NP, 128], [2 * O, 4], [1, O]])
            denv = bass.AP(tensor=nall.tensor, offset=nall.offset + O,
                           ap=[[NP, 128], [2 * O, 4], [1, O]])
            nc.vector.reciprocal(
                rsb.rearrange("p (j o) -> p j o", o=O), denv)
            nc.vector.tensor_tensor(
                out=osb.rearrange("p (j o) -> p j o", o=O),
                in0=numv, in1=rsb.rearrange("p (j o) -> p j o", o=O),
                op=ALU.mult)
            nc.sync.dma_start(
                out=out.rearrange("(j p) o -> p j o", p=128),
                in_=osb.rearrange("p (j o) -> p j o", o=O))

    nc.compile()
    return nc


def build_in_maps(x, train_X, Y, W1, W2, h):
    import jax.numpy as jnp

    def bf(a):
        return np.asarray(jnp.asarray(a, dtype=jnp.bfloat16))

    cbase = _host_consts(float(h))
    x = np.ascontiguousarray(x, dtype=np.float32)
    train_X = np.ascontiguousarray(train_X, dtype=np.float32)
    Y = np.ascontiguousarray(Y, dtype=np.float32)
    W1 = np.ascontiguousarray(W1, dtype=np.float32)
    W2 = np.ascontiguousarray(W2, dtype=np.float32)

    pmod = np.arange(128) % O
    wpack = np.zeros((128, 256), np.float32)
    wpack[0:64, 0:128] = W1.T
    wpack[64:128, 0:128] = W1.T
    wpack[:, 128:256] = W2[pmod].T      # W2rep[k, p] = W2[p%10, k]
    wpack16 = bf(wpack)

    idx = np.arange(N)
    in_maps = []
    for c in range(NCORES):
        n_list = (idx + c * BQ) % N     # core's own queries first
        Xp = train_X[n_list]            # [N, 64]
        Yp = Y[n_list][:, pmod]         # [N, 128]
        m = {"wpack": wpack16}
        for q in range(NPAIR):
            h0 = slice(q * 512, (q + 1) * 512)
            h1 = slice(2048 + q * 512, 2048 + (q + 1) * 512)
            m[f"tX{q}"] = bf(np.concatenate([Xp[h0].T, Xp[h1].T], axis=0))
            m[f"Yr{q}"] = bf(np.concatenate([Yp[h0].T, Yp[h1].T], axis=1))
        cpack = np.zeros((128, 102), np.float32)
        cpack[:, 0:22] = cbase
        cpack[:, 22:102] = _host_yext(Y, c)
        m["cpack"] = cpack
        in_maps.append(m)
    return in_maps


def kernel(x, train_X, Y, W1, W2, h):
    import concourse.bass_utils as bass_utils

    hval = float(h)
    key = ("fgt3", hval)
    if key not in _cache:
        _cache[key] = _build(hval)
    nc = _cache[key]

    in_maps = build_in_maps(x, train_X, Y, W1, W2, h)
    res = bass_utils.run_bass_kernel_spmd(nc, in_maps, core_ids=list(range(NCORES)))
    return np.concatenate([res.results[c]["out"] for c in range(NCORES)], axis=0)


# revision 21
# speedup vs baseline: 1.1393x; 1.1393x over previous
"""Trainium2 Bass kernel for leave-one-out Nadaraya-Watson regression
(nn_Net_72877005078649) — fast-Gauss-transform formulation, v3.

Per output channel o this is 1D Gaussian kernel regression; the kernel
factorizes through a G=12 grid (a = b = h/sqrt(2), trapezoid aliasing
~1e-4):  K_h(x,z) ~= kappa * sum_g exp(-(c_g-x)^2/h^2) exp(-(z-c_g)^2/h^2)

v3 design notes (vs v2 baseline at ~41.5us):
 - per-core input roll: core c's train data is rotated so its own 512
   queries are train chunk 0 — the query-side Eq is just cols 0:512 of
   pair-0's train-side exp table ET0. The whole separate query
   projection chain (W1/relu/W2/sub/sq/exp on [*,512]) is gone.
 - inputs ship packed on 128 partitions, pair-contiguous: tX_q
   [128,512] bf16 holds two 512-col train chunks stacked on partition
   halves (W1T duplicated on rows 0:64 and 64:128 of wpack so both
   halves matmul with base-partition-aligned lhsT). Few large
   contiguous DMAs across 4 engine queues instead of many tiny-packet
   strided ones.
 - finalize: fin_j [128q,20] = matmul(lhsT=ET0[:, j*128:+128] bf16,
   rhs=AA bf16) gives num|den directly in query-partition layout — no
   identity transposes, no PSUM->SBUF copy, no make_identity.
 - elementwise rebalance per pair across ACT/DVE/GpSimd; ACT exp keeps
   den via accum_out, num via STT (DVE on odd / GpSimd on even pairs).
 - diagonal: train_X == x by construction, K_ii == 1 exactly:
   out = (num - Y_d)/(den - 1).

Sharding: queries split across 8 cores (512/core); train replicated.
"""

import numpy as np

N = 4096
D = 64
HID = 128
O = 10
NCORES = 8
BQ = N // NCORES
G = 12
GRID_LO = -6.5
GRID_HI = 6.5
NPAIR = 4               # train chunk pairs, 1024 cols each

_cache = {}


def _host_consts(h: float):
    c = np.linspace(GRID_LO, GRID_HI, G).astype(np.float32)
    delta = float(c[1] - c[0])
    kappa = 2.0 * delta / (np.sqrt(2.0 * np.pi) * h)
    # cbase[128, 22] = cq[128,1] | kmask2[128,20] | -cq[128,1]
    # ET is computed via Derivative_Erf = (2/sqrt(pi))*exp(-u^2); both the
    # query and train factors carry 2/sqrt(pi), so fold (pi/4) into kappa.
    kap = kappa * np.pi / 4.0
    cbase = np.zeros((128, 22), np.float32)
    for p in range(128):
        cbase[p, 0] = c[min(p // O, G - 1)]
        cbase[p, 21] = -c[min(p // O, G - 1)] / h   # DErf bias: -c_g/h
    for p in range(G * O):
        cbase[p, 1 + p % O] = kap            # num mask
        cbase[p, 11 + p % O] = kap           # den mask
    return cbase


def _host_yext(Y, c):
    # yext[p, j*20+e] = Y[c*512 + j*128 + p, e] for e<10, 1.0 for e>=10;
    # lets the finalize subtract num-Y and den-1 in a single tensor_tensor.
    yext = np.ones((128, 4, 2 * O), np.float32)
    yext[:, :, 0:O] = Y[c * BQ:(c + 1) * BQ].reshape(4, 128, O).transpose(1, 0, 2)
    return yext.reshape(128, 4 * 2 * O)


def _build(h: float):
    import concourse.bass as bass
    import concourse.bacc as bacc
    import concourse.tile as tile
    from concourse import mybir

    f32 = mybir.dt.float32
    bf16 = mybir.dt.bfloat16
    AF = mybir.ActivationFunctionType
    ALU = mybir.AluOpType

    inv_h = 1.0 / h

    nc = bacc.Bacc("TRN2", target_bir_lowering=False, debug=False, num_devices=1)
    tXd = [nc.dram_tensor(f"tX{q}", [128, 512], bf16, kind="ExternalInput").ap()
           for q in range(NPAIR)]
    Yrd = [nc.dram_tensor(f"Yr{q}", [128, 1024], bf16, kind="ExternalInput").ap()
           for q in range(NPAIR)]
    wpd = nc.dram_tensor("wpack", [128, 256], bf16, kind="ExternalInput").ap()
    cpd = nc.dram_tensor("cpack", [128, 102], f32, kind="ExternalInput").ap()
    out = nc.dram_tensor("out", [BQ, O], f32, kind="ExternalOutput").ap()

    with tile.TileContext(nc) as tc:
        with (
            tc.tile_pool(name="S", bufs=1) as S,
            tc.tile_pool(name="W", bufs=2) as W,
            tc.tile_pool(name="PS", bufs=1, space="PSUM") as PS,
        ):
            # ---- ACT warmup (loads the multi-func table once) ----
            warm = S.tile([1, 16], f32)
            nc.vector.memset(warm, 0.0)
            nc.scalar.activation(out=warm, in_=warm, func=AF.Derivative_Erf)

            # ---- input DMAs: spread across engine queues, big+contiguous
            wp = S.tile([128, 256], bf16)
            cp = S.tile([128, 102], f32)
            tX = [S.tile([128, 512], bf16, name=f"tX{q}") for q in range(NPAIR)]
            Yr = [S.tile([128, 1024], bf16, name=f"Yr{q}") for q in range(NPAIR)]
            # within-queue FIFO = strict priority; rings fair-share ~300GB/s
            # at packet granularity, so keep every Yr behind all tX chunks
            nc.sync.dma_start(out=tX[0], in_=tXd[0])
            nc.scalar.dma_start(out=wp, in_=wpd)
            nc.gpsimd.dma_start(out=tX[3], in_=tXd[3])
            nc.sync.dma_start(out=tX[1], in_=tXd[1])
            nc.scalar.dma_start(out=cp, in_=cpd)
            nc.scalar.dma_start(out=tX[2], in_=tXd[2])
            nc.sync.dma_start(out=Yr[0], in_=Yrd[0])
            nc.gpsimd.dma_start(out=Yr[2], in_=Yrd[2])
            nc.scalar.dma_start(out=Yr[1], in_=Yrd[1])
            nc.gpsimd.dma_start(out=Yr[3], in_=Yrd[3])

            w1a = wp[0:64, 0:128]
            w1b = wp[64:128, 0:128]
            w2r = wp[:, 128:256]
            kmask2 = cp[:, 1:21]
            ncq = cp[:, 21:22]
            yext = cp[:, 22:102]

            nparts = S.tile([128, NPAIR + 1], f32)
            dparts = S.tile([128, NPAIR + 1], f32)
            parts = S.tile([128, 2], f32)
            AA = S.tile([128, 2 * O], bf16)
            ET0 = S.tile([128, 1024], bf16)
            nall = S.tile([128, 4 * 2 * O], f32)
            rsb = S.tile([128, 4 * O], f32)
            osb = S.tile([128, 4 * O], f32)

            # ---- PE warmup: dummy matmuls during the DMA wait trigger the
            # HAM clock boost before the real pipeline starts
            wps = PS.tile([128, 1024], f32, tag="hps", bufs=2, name="wps")
            for _ in range(12):
                nc.tensor.matmul(wps[0:16, 0:16], lhsT=warm[0:1, 0:16],
                                 rhs=warm[0:1, 0:16], start=True, stop=True)

            # ---- 4 train pairs, software-pipelined ----
            hps_t = []

            def w1_pair(q):
                hps = PS.tile([128, 1024], f32, tag="hps", bufs=2,
                              name=f"hps{q}")
                hps_t.append(hps)
                nc.tensor.matmul(hps[:, 0:512], lhsT=w1a, rhs=tX[q][0:64, :],
                                 start=True, stop=True)
                nc.tensor.matmul(hps[:, 512:1024], lhsT=w1b,
                                 rhs=tX[q][64:128, :],
                                 start=True, stop=True)

            w1_pair(0)
            for q in range(NPAIR):
                last = q == NPAIR - 1
                h1 = W.tile([128, 1024], bf16, tag="h1", bufs=3)
                if last:
                    # split halves: shorter serial drain into the finalize
                    nc.vector.tensor_scalar(out=h1[:, 0:512],
                                            in0=hps_t[q][:, 0:512],
                                            scalar1=0.0, scalar2=None,
                                            op0=ALU.max)
                    nc.vector.tensor_scalar(out=h1[:, 512:1024],
                                            in0=hps_t[q][:, 512:1024],
                                            scalar1=0.0, scalar2=None,
                                            op0=ALU.max)
                elif q == 1:
                    # one relu on ACT to balance engine load
                    nc.scalar.activation(out=h1, in_=hps_t[q], func=AF.Relu)
                else:
                    nc.vector.tensor_scalar(out=h1, in0=hps_t[q], scalar1=0.0,
                                            scalar2=None, op0=ALU.max)
                if q + 1 < NPAIR:
                    w1_pair(q + 1)
                xr = PS.tile([128, 1024], f32, tag="xr", bufs=2, name=f"xr{q}")
                nc.tensor.matmul(xr[:, 0:512], lhsT=w2r, rhs=h1[:, 0:512],
                                 start=True, stop=True)
                nc.tensor.matmul(xr[:, 512:1024], lhsT=w2r, rhs=h1[:, 512:1024],
                                 start=True, stop=True)
                # fused Gaussian: DErf(xr/h - c_g/h) = 2/sqrt(pi) exp(-s(xr-c)^2)
                ET = ET0 if q == 0 else W.tile([128, 1024], bf16, tag="ET",
                                               bufs=3)
                scr = W.tile([128, 1024], bf16, tag="scr", bufs=3)
                if last:
                    for hh in range(2):
                        sl = slice(hh * 512, (hh + 1) * 512)
                        nc.scalar.activation(out=ET[:, sl], in_=xr[:, sl],
                                             func=AF.Derivative_Erf,
                                             bias=ncq, scale=inv_h,
                                             accum_out=dparts[:, q + hh:q + hh + 1])
                        nc.vector.scalar_tensor_tensor(
                            out=scr[:, sl], in0=ET[:, sl], scalar=1.0,
                            in1=Yr[q][:, sl],
                            op0=ALU.bypass, op1=ALU.mult,
                            accum_out=nparts[:, q + hh:q + hh + 1])
                else:
                    nc.scalar.activation(out=ET, in_=xr,
                                         func=AF.Derivative_Erf,
                                         bias=ncq, scale=inv_h,
                                         accum_out=dparts[:, q:q + 1])
                    nc.vector.scalar_tensor_tensor(
                        out=scr, in0=ET, scalar=1.0, in1=Yr[q],
                        op0=ALU.bypass, op1=ALU.mult,
                        accum_out=nparts[:, q:q + 1])

            # ---- tables -> AA (bf16 for the bf16 fin matmuls) ----
            nc.vector.tensor_reduce(out=parts[:, 0:1], in_=nparts,
                                    axis=mybir.AxisListType.X, op=ALU.add)
            nc.vector.tensor_reduce(out=parts[:, 1:2], in_=dparts,
                                    axis=mybir.AxisListType.X, op=ALU.add)
            PP = parts.ap[0][0]
            parts_b = bass.AP(tensor=parts.tensor, offset=parts.offset,
                              ap=[[PP, 128], [1, 2], [0, O]])
            nc.vector.tensor_tensor(out=AA.rearrange("p (k e) -> p k e", e=O),
                                    in0=parts_b,
                                    in1=kmask2.rearrange("p (k e) -> p k e", e=O),
                                    op=ALU.mult)

            # ---- query contraction directly into query-partition layout
            fin = PS.tile([128, 4 * 2 * O], f32, tag="xr", bufs=2, name="fin")
            for j in range(4):
                nc.tensor.matmul(fin[:, j * 20:(j + 1) * 20],
                                 lhsT=ET0[:, j * 128:(j + 1) * 128], rhs=AA,
                                 start=True, stop=True)

            # single subtract: num-Y and den-1 at once (yext = Y | ones)
            nc.vector.tensor_tensor(out=nall, in0=fin, in1=yext,
                                    op=ALU.subtract)
            NP = nall.ap[0][0]
            numv = bass.AP(tensor=nall.tensor, offset=nall.offset,
                           ap=[[NP, 128], [2 * O, 4], [1, O]])
            denv = bass.AP(tensor=nall.tensor, offset=nall.offset + O,
                           ap=[[NP, 128], [2 * O, 4], [1, O]])
            nc.vector.reciprocal(
                rsb.rearrange("p (j o) -> p j o", o=O), denv)
            nc.vector.tensor_tensor(
                out=osb.rearrange("p (j o) -> p j o", o=O),
                in0=numv, in1=rsb.rearrange("p (j o) -> p j o", o=O),
                op=ALU.mult)
            nc.sync.dma_start(
                out=out.rearrange("(j p) o -> p j o", p=128),
                in_=osb.rearrange("p (j o) -> p j o", o=O))

    nc.compile()
    return nc


def build_in_maps(x, train_X, Y, W1, W2, h):
    import jax.numpy as jnp

    def bf(a):
        return np.asarray(jnp.asarray(a, dtype=jnp.bfloat16))

    cbase = _host_consts(float(h))
    x = np.ascontiguousarray(x, dtype=np.float32)
    train_X = np.ascontiguousarray(train_X, dtype=np.float32)
    Y = np.ascontiguousarray(Y, dtype=np.float32)
    W1 = np.ascontiguousarray(W1, dtype=np.float32)
    W2 = np.ascontiguousarray(W2, dtype=np.float32)

    pmod = np.arange(128) % O
    wpack = np.zeros((128, 256), np.float32)
    wpack[0:64, 0:128] = W1.T
    wpack[64:128, 0:128] = W1.T
    wpack[:, 128:256] = W2[pmod].T      # W2rep[k, p] = W2[p%10, k]
    wpack16 = bf(wpack)

    idx = np.arange(N)
    in_maps = []
    for c in range(NCORES):
        n_list = (idx + c * BQ) % N     # core's own queries first
        Xp = train_X[n_list]            # [N, 64]
        Yp = Y[n_list][:, pmod]         # [N, 128]
        m = {"wpack": wpack16}
        for q in range(NPAIR):
            h0 = slice(q * 512, (q + 1) * 512)
            h1 = slice(2048 + q * 512, 2048 + (q + 1) * 512)
            m[f"tX{q}"] = bf(np.concatenate([Xp[h0].T, Xp[h1].T], axis=0))
            m[f"Yr{q}"] = bf(np.concatenate([Yp[h0].T, Yp[h1].T], axis=1))
        cpack = np.zeros((128, 102), np.float32)
        cpack[:, 0:22] = cbase
        cpack[:, 22:102] = _host_yext(Y, c)
        m["cpack"] = cpack
        in_maps.append(m)
    return in_maps


def kernel(x, train_X, Y, W1, W2, h):
    import concourse.bass_utils as bass_utils

    hval = float(h)
    key = ("fgt3", hval)
    if key not in _cache:
        _cache[key] = _build(hval)
    nc = _cache[key]

    in_maps = build_in_maps(x, train_X, Y, W1, W2, h)
    res = bass_utils.run_bass_kernel_spmd(nc, in_maps, core_ids=list(range(NCORES)))
    return np.concatenate([res.results[c]["out"] for c in range(NCORES)], axis=0)


# revision 25
# speedup vs baseline: 1.2651x; 1.1105x over previous
"""Trainium2 Bass kernel for leave-one-out Nadaraya-Watson regression
(nn_Net_72877005078649) — fast-Gauss-transform formulation, v3.

Per output channel o this is 1D Gaussian kernel regression; the kernel
factorizes through a G=12 grid (a = b = h/sqrt(2), trapezoid aliasing
~1e-4):  K_h(x,z) ~= kappa * sum_g exp(-(c_g-x)^2/h^2) exp(-(z-c_g)^2/h^2)

v3 design notes (vs v2 baseline at ~41.5us):
 - per-core input roll: core c's train data is rotated so its own 512
   queries are train chunk 0 — the query-side Eq is just cols 0:512 of
   pair-0's train-side exp table ET0. The whole separate query
   projection chain (W1/relu/W2/sub/sq/exp on [*,512]) is gone.
 - inputs ship packed on 128 partitions, pair-contiguous: tX_q
   [128,512] bf16 holds two 512-col train chunks stacked on partition
   halves (W1T duplicated on rows 0:64 and 64:128 of wpack so both
   halves matmul with base-partition-aligned lhsT). Few large
   contiguous DMAs across 4 engine queues instead of many tiny-packet
   strided ones.
 - finalize: fin_j [128q,20] = matmul(lhsT=ET0[:, j*128:+128] bf16,
   rhs=AA bf16) gives num|den directly in query-partition layout — no
   identity transposes, no PSUM->SBUF copy, no make_identity.
 - elementwise rebalance per pair across ACT/DVE/GpSimd; ACT exp keeps
   den via accum_out, num via STT (DVE on odd / GpSimd on even pairs).
 - diagonal: train_X == x by construction, K_ii == 1 exactly:
   out = (num - Y_d)/(den - 1).

Sharding: queries split across 8 cores (512/core); train replicated.
"""

import numpy as np

N = 4096
D = 64
HID = 128
O = 10
NCORES = 8
BQ = N // NCORES
G = 12
GRID_LO = -6.5
GRID_HI = 6.5
NPAIR = 4               # train chunk pairs, 1024 cols each

_cache = {}


def _host_consts(h: float):
    c = np.linspace(GRID_LO, GRID_HI, G).astype(np.float32)
    delta = float(c[1] - c[0])
    kappa = 2.0 * delta / (np.sqrt(2.0 * np.pi) * h)
    # cbase[128, 22] = cq[128,1] | kmask2[128,20] | -cq[128,1]
    # ET is computed via Derivative_Erf = (2/sqrt(pi))*exp(-u^2); both the
    # query and train factors carry 2/sqrt(pi), so fold (pi/4) into kappa.
    kap = kappa * np.pi / 4.0
    cbase = np.zeros((128, 22), np.float32)
    for p in range(128):
        cbase[p, 0] = c[min(p // O, G - 1)]
        cbase[p, 21] = -c[min(p // O, G - 1)] / h   # DErf bias: -c_g/h
    for p in range(G * O):
        cbase[p, 1 + p % O] = kap            # num mask
        cbase[p, 11 + p % O] = kap           # den mask
    return cbase


def _host_yext(Y, c):
    # yext[p, j*20+e] = Y[c*512 + j*128 + p, e] for e<10, 1.0 for e>=10;
    # lets the finalize subtract num-Y and den-1 in a single tensor_tensor.
    yext = np.ones((128, 4, 2 * O), np.float32)
    yext[:, :, 0:O] = Y[c * BQ:(c + 1) * BQ].reshape(4, 128, O).transpose(1, 0, 2)
    return yext.reshape(128, 4 * 2 * O)


def _build(h: float):
    import concourse.bass as bass
    import concourse.bacc as bacc
    import concourse.tile as tile
    from concourse import mybir

    f32 = mybir.dt.float32
    bf16 = mybir.dt.bfloat16
    AF = mybir.ActivationFunctionType
    ALU = mybir.AluOpType

    inv_h = 1.0 / h

    nc = bacc.Bacc("TRN2", target_bir_lowering=False, debug=False, num_devices=1)
    tXd = [nc.dram_tensor(f"tX{q}", [128, 512], bf16, kind="ExternalInput").ap()
           for q in range(NPAIR)]
    Yrd = [nc.dram_tensor(f"Yr{q}", [128, 1024], bf16, kind="ExternalInput").ap()
           for q in range(NPAIR)]
    wpd = nc.dram_tensor("wpack", [128, 256], bf16, kind="ExternalInput").ap()
    cpd = nc.dram_tensor("cpack", [128, 102], f32, kind="ExternalInput").ap()
    out = nc.dram_tensor("out", [BQ, O], f32, kind="ExternalOutput").ap()

    with tile.TileContext(nc) as tc:
        with (
            tc.tile_pool(name="S", bufs=1) as S,
            tc.tile_pool(name="W", bufs=2) as W,
            tc.tile_pool(name="PS", bufs=1, space="PSUM") as PS,
        ):
            # ---- warmup tiles: wmm feeds PE warmup (vector memset only, no
            # ACT dependency); warm triggers the ACT table load early
            wmm = S.tile([1, 16], f32)
            nc.vector.memset(wmm, 0.0)
            warm = S.tile([1, 16], f32)
            nc.vector.memset(warm, 0.0)
            nc.scalar.activation(out=warm, in_=warm, func=AF.Derivative_Erf)

            # ---- input DMAs: spread across engine queues, big+contiguous
            wp = S.tile([128, 256], bf16)
            cp = S.tile([128, 102], f32)
            tX = [S.tile([128, 512], bf16, name=f"tX{q}") for q in range(NPAIR)]
            Yr = [S.tile([128, 1024], bf16, name=f"Yr{q}") for q in range(NPAIR)]
            # within-queue FIFO = strict priority; rings fair-share ~300GB/s
            # at packet granularity. Scalar engine issues NO DMAs (its queue
            # must stay free for ACT table loads + activations).
            nc.sync.dma_start(out=tX[0], in_=tXd[0])
            nc.gpsimd.dma_start(out=wp, in_=wpd)
            nc.sync.dma_start(out=tX[1], in_=tXd[1])
            nc.gpsimd.dma_start(out=cp, in_=cpd)
            nc.gpsimd.dma_start(out=tX[2], in_=tXd[2])
            nc.gpsimd.dma_start(out=tX[3], in_=tXd[3])
            nc.sync.dma_start(out=Yr[0], in_=Yrd[0])
            nc.sync.dma_start(out=Yr[1], in_=Yrd[1])
            nc.gpsimd.dma_start(out=Yr[2], in_=Yrd[2])
            nc.gpsimd.dma_start(out=Yr[3], in_=Yrd[3])

            w1a = wp[0:64, 0:128]
            w1b = wp[64:128, 0:128]
            w2r = wp[:, 128:256]
            kmask2 = cp[:, 1:21]
            ncq = cp[:, 21:22]
            yext = cp[:, 22:102]

            nparts = S.tile([128, NPAIR + 1], f32)
            dparts = S.tile([128, NPAIR + 1], f32)
            parts = S.tile([128, 2], f32)
            AA = S.tile([128, 2 * O], bf16)
            ET0 = S.tile([128, 1024], bf16)
            nall = S.tile([128, 4 * 2 * O], f32)
            rsb = S.tile([128, 4 * O], f32)
            osb = S.tile([128, 4 * O], f32)

            # ---- PE warmup: dummy matmuls during the DMA wait trigger the
            # HAM clock boost before the real pipeline starts
            wps = PS.tile([128, 1024], f32, tag="hps", bufs=2, name="wps")
            for _ in range(12):
                nc.tensor.matmul(wps[0:16, 0:16], lhsT=wmm[0:1, 0:16],
                                 rhs=wmm[0:1, 0:16], start=True, stop=True)

            # ---- 4 train pairs, software-pipelined ----
            hps_t = []

            def w1_pair(q):
                hps = PS.tile([128, 1024], f32, tag="hps", bufs=2,
                              name=f"hps{q}")
                hps_t.append(hps)
                nc.tensor.matmul(hps[:, 0:512], lhsT=w1a, rhs=tX[q][0:64, :],
                                 start=True, stop=True)
                nc.tensor.matmul(hps[:, 512:1024], lhsT=w1b,
                                 rhs=tX[q][64:128, :],
                                 start=True, stop=True)

            w1_pair(0)
            for q in range(NPAIR):
                last = q == NPAIR - 1
                h1 = W.tile([128, 1024], bf16, tag="h1", bufs=3)
                if last:
                    # split halves on ACT: shorter serial drain into finalize
                    nc.scalar.activation(out=h1[:, 0:512],
                                         in_=hps_t[q][:, 0:512], func=AF.Relu)
                    nc.scalar.activation(out=h1[:, 512:1024],
                                         in_=hps_t[q][:, 512:1024],
                                         func=AF.Relu)
                elif q == 1:
                    nc.scalar.activation(out=h1, in_=hps_t[q], func=AF.Relu)
                else:
                    nc.vector.tensor_scalar(out=h1, in0=hps_t[q], scalar1=0.0,
                                            scalar2=None, op0=ALU.max)
                if q + 1 < NPAIR:
                    w1_pair(q + 1)
                xr = PS.tile([128, 1024], f32, tag="xr", bufs=2, name=f"xr{q}")
                nc.tensor.matmul(xr[:, 0:512], lhsT=w2r, rhs=h1[:, 0:512],
                                 start=True, stop=True)
                nc.tensor.matmul(xr[:, 512:1024], lhsT=w2r, rhs=h1[:, 512:1024],
                                 start=True, stop=True)
                # fused Gaussian: DErf(xr/h - c_g/h) = 2/sqrt(pi) exp(-s(xr-c)^2)
                ET = ET0 if q == 0 else W.tile([128, 1024], bf16, tag="ET",
                                               bufs=3)
                scr = W.tile([128, 1024], bf16, tag="scr", bufs=3)
                if last:
                    for hh in range(2):
                        sl = slice(hh * 512, (hh + 1) * 512)
                        nc.scalar.activation(out=ET[:, sl], in_=xr[:, sl],
                                             func=AF.Derivative_Erf,
                                             bias=ncq, scale=inv_h,
                                             accum_out=dparts[:, q + hh:q + hh + 1])
                        nc.vector.scalar_tensor_tensor(
                            out=scr[:, sl], in0=ET[:, sl], scalar=1.0,
                            in1=Yr[q][:, sl],
                            op0=ALU.bypass, op1=ALU.mult,
                            accum_out=nparts[:, q + hh:q + hh + 1])
                else:
                    nc.scalar.activation(out=ET, in_=xr,
                                         func=AF.Derivative_Erf,
                                         bias=ncq, scale=inv_h,
                                         accum_out=dparts[:, q:q + 1])
                    nc.vector.scalar_tensor_tensor(
                        out=scr, in0=ET, scalar=1.0, in1=Yr[q],
                        op0=ALU.bypass, op1=ALU.mult,
                        accum_out=nparts[:, q:q + 1])

            # ---- tables -> AA (bf16 for the bf16 fin matmuls) ----
            nc.vector.tensor_reduce(out=parts[:, 0:1], in_=nparts,
                                    axis=mybir.AxisListType.X, op=ALU.add)
            nc.vector.tensor_reduce(out=parts[:, 1:2], in_=dparts,
                                    axis=mybir.AxisListType.X, op=ALU.add)
            PP = parts.ap[0][0]
            parts_b = bass.AP(tensor=parts.tensor, offset=parts.offset,
                              ap=[[PP, 128], [1, 2], [0, O]])
            nc.vector.tensor_tensor(out=AA.rearrange("p (k e) -> p k e", e=O),
                                    in0=parts_b,
                                    in1=kmask2.rearrange("p (k e) -> p k e", e=O),
                                    op=ALU.mult)

            # ---- query contraction directly into query-partition layout
            fin = PS.tile([128, 4 * 2 * O], f32, tag="xr", bufs=2, name="fin")
            for j in range(4):
                nc.tensor.matmul(fin[:, j * 20:(j + 1) * 20],
                                 lhsT=ET0[:, j * 128:(j + 1) * 128], rhs=AA,
                                 start=True, stop=True)

            # single subtract: num-Y and den-1 at once (yext = Y | ones)
            nc.vector.tensor_tensor(out=nall, in0=fin, in1=yext,
                                    op=ALU.subtract)
            NP = nall.ap[0][0]
            numv = bass.AP(tensor=nall.tensor, offset=nall.offset,
                           ap=[[NP, 128], [2 * O, 4], [1, O]])
            denv = bass.AP(tensor=nall.tensor, offset=nall.offset + O,
                           ap=[[NP, 128], [2 * O, 4], [1, O]])
            nc.vector.reciprocal(
                rsb.rearrange("p (j o) -> p j o", o=O), denv)
            nc.vector.tensor_tensor(
                out=osb.rearrange("p (j o) -> p j o", o=O),
                in0=numv, in1=rsb.rearrange("p (j o) -> p j o", o=O),
                op=ALU.mult)
            nc.sync.dma_start(
                out=out.rearrange("(j p) o -> p j o", p=128),
                in_=osb.rearrange("p (j o) -> p j o", o=O))

    nc.compile()
    return nc


def build_in_maps(x, train_X, Y, W1, W2, h):
    import jax.numpy as jnp

    def bf(a):
        return np.asarray(jnp.asarray(a, dtype=jnp.bfloat16))

    cbase = _host_consts(float(h))
    x = np.ascontiguousarray(x, dtype=np.float32)
    train_X = np.ascontiguousarray(train_X, dtype=np.float32)
    Y = np.ascontiguousarray(Y, dtype=np.float32)
    W1 = np.ascontiguousarray(W1, dtype=np.float32)
    W2 = np.ascontiguousarray(W2, dtype=np.float32)

    pmod = np.arange(128) % O
    wpack = np.zeros((128, 256), np.float32)
    wpack[0:64, 0:128] = W1.T
    wpack[64:128, 0:128] = W1.T
    wpack[:, 128:256] = W2[pmod].T      # W2rep[k, p] = W2[p%10, k]
    wpack16 = bf(wpack)

    idx = np.arange(N)
    in_maps = []
    for c in range(NCORES):
        n_list = (idx + c * BQ) % N     # core's own queries first
        Xp = train_X[n_list]            # [N, 64]
        Yp = Y[n_list][:, pmod]         # [N, 128]
        m = {"wpack": wpack16}
        for q in range(NPAIR):
            h0 = slice(q * 512, (q + 1) * 512)
            h1 = slice(2048 + q * 512, 2048 + (q + 1) * 512)
            m[f"tX{q}"] = bf(np.concatenate([Xp[h0].T, Xp[h1].T], axis=0))
            m[f"Yr{q}"] = bf(np.concatenate([Yp[h0].T, Yp[h1].T], axis=1))
        cpack = np.zeros((128, 102), np.float32)
        cpack[:, 0:22] = cbase
        cpack[:, 22:102] = _host_yext(Y, c)
        m["cpack"] = cpack
        in_maps.append(m)
    return in_maps


def kernel(x, train_X, Y, W1, W2, h):
    import concourse.bass_utils as bass_utils

    hval = float(h)
    key = ("fgt3", hval)
    if key not in _cache:
        _cache[key] = _build(hval)
    nc = _cache[key]

    in_maps = build_in_maps(x, train_X, Y, W1, W2, h)
    res = bass_utils.run_bass_kernel_spmd(nc, in_maps, core_ids=list(range(NCORES)))
    return np.concatenate([res.results[c]["out"] for c in range(NCORES)], axis=0)
